# revision 20
# baseline (speedup 1.0000x reference)
"""CPAMDec attention-decoder kernel for 8 Trainium2 NeuronCores.

Reference computation (per batch n of N=8):
    q  = x_n^T @ wq^T + bq          (HW=4096, C4=128)
    k  = y_n @ wk^T + bk            (K=32, C4=128)
    v  = y_n @ wv^T + bv            (K=32, C=512)
    attn = softmax(q @ k^T, axis=-1)        (HW, K)
    out = scale * (v^T @ attn^T) + x_n      (C, HW)

Sharding: pure data parallel — core i computes batch i. Params are
replicated (host pre-transposes them so no on-device transposes are
needed). All heavy matmuls run in float32r (1 cycle/row at N=512).
The residual add reads the original fp32 bits of x, so the dominant
output term is exact.

Bias folding:
  - bq is folded into a per-partition bias e_b[j] = sum_o bq[o]*k[j,o]
    applied inside the exp() activation (softmax is shift-invariant in
    q-space: energy row bias only -> actually exact algebra:
    q@k^T = q0@k^T + bq@k^T, a per-j column offset).
  - bv is folded into an extra (33rd) contraction row of the final
    matmul: v_aug row 32 = scale*bv, attn_aug row 32 = 1.0, using
    sum_j attn[p,j] = 1.
"""

import sys

sys.path.insert(0, "/opt/trn_rl_repo")

import numpy as np

import concourse.bacc as bacc
import concourse.mybir as mybir
import concourse.tile as tile
from concourse.bass_utils import run_bass_kernel_spmd

F32 = mybir.dt.float32
F32R = mybir.dt.float32r
AF = mybir.ActivationFunctionType

N, C, H, W, K = 8, 512, 64, 64, 32
HW = H * W            # 4096
C4 = C // 4           # 128
PC = 512              # free-dim chunk (1 PSUM bank of fp32)
NPC = HW // PC        # 8 chunks
KC = C // 128         # 4 contraction chunks
CT = C // 128         # 4 output row-tiles


def _emit(nc, tc):
    sync = nc.sync
    cdma = nc.scalar  # second HWDGE ring — constants go here

    with (
        tc.tile_pool(name="const", bufs=1) as cst,
        tc.tile_pool(name="xbuf", bufs=1) as xp,
        tc.tile_pool(name="big", bufs=1) as big,
        tc.tile_pool(name="work", bufs=4) as wk_pool,
        tc.tile_pool(name="ps", bufs=8, space="PSUM") as ps,
    ):
        # ---------------- x loads first (big, 2 MB each, critical path) ----
        xs = []
        for k in range(KC):
            t = xp.tile([128, HW], F32R, name=f"xs{k}", tag=f"xs{k}")
            sync.dma_start(t[:], nc.t.x[k * 128:(k + 1) * 128, :].bitcast(F32R))
            xs.append(t)

        # ---------------- constant loads (scalar HWDGE ring) ----------------
        yt = []
        wq = []
        wkt = []
        wv = []
        for k in range(KC):
            t = cst.tile([128, K], F32R, name=f"yt{k}", tag=f"yt{k}")
            cdma.dma_start(t[:], nc.t.yT[k * 128:(k + 1) * 128, :].bitcast(F32R))
            yt.append(t)
        for k in range(KC):
            t = cst.tile([128, C4], F32R, name=f"wq{k}", tag=f"wq{k}")
            cdma.dma_start(t[:], nc.t.wqT[k * 128:(k + 1) * 128, :].bitcast(F32R))
            wq.append(t)
        for k in range(KC):
            t = cst.tile([128, C4], F32R, name=f"wk{k}", tag=f"wk{k}")
            cdma.dma_start(t[:], nc.t.wkT[k * 128:(k + 1) * 128, :].bitcast(F32R))
            wkt.append(t)
        for k in range(KC):
            t = cst.tile([128, C], F32R, name=f"wv{k}", tag=f"wv{k}")
            cdma.dma_start(t[:], nc.t.wvT[k * 128:(k + 1) * 128, :].bitcast(F32R))
            wv.append(t)

        bq_r = cst.tile([C4, K], F32R, name="bq_r", tag="bq_r")
        cdma.dma_start(bq_r[:], nc.t.bqb[:].bitcast(F32R))
        bk_sb = cst.tile([C4, 1], F32, name="bk_sb", tag="bk_sb")
        cdma.dma_start(bk_sb[:], nc.t.bk[:])
        bv_row = cst.tile([1, C], F32, name="bv_row", tag="bv_row")
        cdma.dma_start(bv_row[:], nc.t.bv[:])
        s_sb1 = cst.tile([1, 1], F32, name="s_sb1", tag="s_sb1")
        cdma.dma_start(s_sb1[:], nc.t.s[:])
        s_bc32 = cst.tile([K, 1], F32, name="s_bc32", tag="s_bc32")
        nc.gpsimd.dma_start(
            s_bc32[:], nc.t.s[:].partition_broadcast(K).squeeze(-1))

        ones32 = cst.tile([K, K], F32R, name="ones32", tag="ones32")
        nc.gpsimd.dma_start(
            ones32[:],
            nc.t.ones[0:1, 0:K].bitcast(F32R).partition_broadcast(K).squeeze(1))

        # ---------------- prologue: kT, v_aug, e_b ----------------
        # kT[o, j] = sum_c wkT[c, o] * yT[c, j]  (+ bk per-partition)
        kt_ps = ps.tile([C4, K], F32, name="kt_ps", tag="ps")
        for k in range(KC):
            nc.tensor.matmul(kt_ps[:], wkt[k][:], yt[k][:],
                             start=(k == 0), stop=(k == KC - 1))
        ktb = cst.tile([C4, K], F32R, name="ktb", tag="ktb")
        nc.scalar.activation(out=ktb[:], in_=kt_ps[:], func=AF.Identity,
                             bias=bk_sb[:], scale=1.0)

        # v_aug rows 0..31 = scale * (y @ wv^T); row 32 = scale * bv
        v_ps = ps.tile([K, C], F32, name="v_ps", tag="ps")
        for k in range(KC):
            nc.tensor.matmul(v_ps[:], yt[k][:], wv[k][:],
                             start=(k == 0), stop=(k == KC - 1))
        v_aug = cst.tile([K + 1, C], F32R, name="v_aug", tag="v_aug")
        nc.scalar.activation(out=v_aug[0:K, :], in_=v_ps[:], func=AF.Copy,
                             bias=0.0, scale=s_bc32[:])
        nc.vector.tensor_scalar_mul(v_aug[K:K + 1, :], bv_row[:], s_sb1[:])

        # e_b[j] = sum_o ktb[o, j] * bq[o]   -> (K, 1) per-partition bias
        eb_ps = ps.tile([K, K], F32, name="eb_ps", tag="ps")
        nc.tensor.matmul(eb_ps[:], ktb[:], bq_r[:], start=True, stop=True)
        e_b = cst.tile([K, 1], F32, name="e_b", tag="e_b")
        nc.scalar.activation(out=e_b[:], in_=eb_ps[:, 0:1], func=AF.Copy,
                             scale=1.0)

        # ---------------- q projection: qT[o, p] ----------------
        qt = big.tile([C4, HW], F32R, name="qt", tag="qt")
        q_ps = [ps.tile([C4, PC], F32, name=f"q_ps{pc}", tag="ps")
                for pc in range(NPC)]
        for k in range(KC):
            for pc in range(NPC):
                nc.tensor.matmul(
                    q_ps[pc][:], wq[k][:],
                    xs[k][:, pc * PC:(pc + 1) * PC],
                    start=(k == 0), stop=(k == KC - 1),
                )
        for pc in range(NPC):
            nc.scalar.activation(out=qt[:, pc * PC:(pc + 1) * PC],
                                 in_=q_ps[pc][:], func=AF.Copy, scale=1.0)

        # ---------------- attention: energy -> softmax ----------------
        expt = big.tile([K, HW], F32R, name="expt", tag="expt")
        attn = big.tile([K + 1, HW], F32R, name="attn", tag="attn")
        sync.dma_start(attn[K:K + 1, :], nc.t.ones[:].bitcast(F32R))

        for pc in range(NPC):
            sl = slice(pc * PC, (pc + 1) * PC)
            e_ps = ps.tile([K, PC], F32, name=f"e_ps{pc}", tag="ps")
            nc.tensor.matmul(e_ps[:], ktb[:], qt[:, sl], start=True, stop=True)
            nc.scalar.activation(out=expt[:, sl], in_=e_ps[:], func=AF.Exp,
                                 bias=e_b[:], scale=1.0)
            s_ps = ps.tile([K, PC], F32, name=f"s_ps{pc}", tag="ps")
            nc.tensor.matmul(s_ps[:], ones32[:], expt[:, sl],
                             start=True, stop=True)
            rec = wk_pool.tile([K, PC], F32, name="rec", tag="rec", bufs=3)
            nc.vector.reciprocal_approx_fast(
                out=rec[:], in_=s_ps[:].bitcast(F32))
            nc.vector.tensor_mul(
                attn[0:K, sl], expt[:, sl].bitcast(F32), rec[:])

        # ---------------- output: U = v_aug^T @ attn_aug; out = U + x ----
        for pc in range(NPC):
            sl = slice(pc * PC, (pc + 1) * PC)
            for ct in range(CT):
                o_ps = ps.tile([128, PC], F32, name=f"o_ps{pc}_{ct}", tag="ps")
                nc.tensor.matmul(o_ps[:], v_aug[:, ct * 128:(ct + 1) * 128],
                                 attn[:, sl], start=True, stop=True)
                osb = wk_pool.tile([128, PC], F32, name="osb", tag="osb",
                                   bufs=8)
                if ct < 2:
                    nc.vector.tensor_add(
                        osb[:], o_ps[:], xs[ct][:, sl].bitcast(F32))
                else:
                    tmp = wk_pool.tile([128, PC], F32, name="tmp", tag="tmp",
                                       bufs=4)
                    nc.scalar.activation(out=tmp[:], in_=o_ps[:],
                                         func=AF.Copy, scale=1.0)
                    nc.gpsimd.tensor_add(
                        osb[:], tmp[:], xs[ct][:, sl].bitcast(F32))
                sync.dma_start(nc.t.out[ct * 128:(ct + 1) * 128, sl], osb[:])


class _T:
    """Attribute access to declared dram params."""
    def __init__(self):
        self.__dict__ = {}


_NC_CACHE = []


def _build():
    if _NC_CACHE:
        return _NC_CACHE[0]
    nc = bacc.Bacc(target_bir_lowering=False)
    nc.t = _T()
    t = nc.t
    t.x = nc.declare_dram_parameter("x", [C, HW], F32, isOutput=False)
    t.yT = nc.declare_dram_parameter("yT", [C, K], F32, isOutput=False)
    t.wqT = nc.declare_dram_parameter("wqT", [C, C4], F32, isOutput=False)
    t.wkT = nc.declare_dram_parameter("wkT", [C, C4], F32, isOutput=False)
    t.wvT = nc.declare_dram_parameter("wvT", [C, C], F32, isOutput=False)
    t.bqb = nc.declare_dram_parameter("bqb", [C4, K], F32, isOutput=False)
    t.bk = nc.declare_dram_parameter("bk", [C4, 1], F32, isOutput=False)
    t.bv = nc.declare_dram_parameter("bv", [1, C], F32, isOutput=False)
    t.s = nc.declare_dram_parameter("s", [1, 1], F32, isOutput=False)
    t.ones = nc.declare_dram_parameter("ones", [1, HW], F32, isOutput=False)
    t.out = nc.declare_dram_parameter("out", [C, HW], F32, isOutput=True)
    with tile.TileContext(nc) as tc:
        _emit(nc, tc)
    nc.finalize()
    _NC_CACHE.append(nc)
    return nc


def _in_maps(x, y, wq, bq, wk, bk, wv, bv, scale):
    x = np.ascontiguousarray(x, dtype=np.float32).reshape(N, C, HW)
    yT = np.ascontiguousarray(np.transpose(y, (0, 2, 1)), dtype=np.float32)
    wqT = np.ascontiguousarray(wq.T, dtype=np.float32)
    wkT = np.ascontiguousarray(wk.T, dtype=np.float32)
    wvT = np.ascontiguousarray(wv.T, dtype=np.float32)
    bqb = np.ascontiguousarray(
        np.broadcast_to(np.float32(bq).reshape(C4, 1), (C4, K)),
        dtype=np.float32)
    bk = np.ascontiguousarray(bk, dtype=np.float32).reshape(C4, 1)
    bv = np.ascontiguousarray(bv, dtype=np.float32).reshape(1, C)
    s = np.ascontiguousarray(scale, dtype=np.float32).reshape(1, 1)
    return [
        {
            "x": x[i], "yT": yT[i], "wqT": wqT, "wkT": wkT, "wvT": wvT,
            "bqb": bqb, "bk": bk, "bv": bv, "s": s,
            "ones": np.ones((1, HW), dtype=np.float32),
        }
        for i in range(N)
    ]


def _run(inputs, **kwargs):
    nc = _build()
    return run_bass_kernel_spmd(nc, _in_maps(**inputs),
                                core_ids=list(range(N)), **kwargs)


def kernel(**inputs) -> np.ndarray:
    res = _run(inputs)
    out = np.stack([res.results[i]["out"] for i in range(N)])
    return out.reshape(N, C, H, W).astype(np.float32)
